# revision 2
# baseline (speedup 1.0000x reference)
"""Trainium2 Bass kernel for speculative-decoding rejection sampling.

kernel(**inputs) takes the FULL inputs (B=128 requests x SPEC=8 draft tokens,
V=32000 vocab) and returns the FULL [128, 9] int32 output. Internally the 128
requests are sharded 16-per-core across 8 NeuronCores (data parallel over
requests); each core keeps full vocab rows for its tokens so argmax / softmax
reductions over vocab stay local. Greedy-request rows only need argmax of the
logits; non-greedy rows only need the softmax denominator; the expensive
"recovered token" argmax over max(p-d,0)/q is computed on-device only for the
single first-rejection row of each non-greedy request, selected with on-device
indirect DMA gathers.
"""
from contextlib import ExitStack

import numpy as np

GREEDY_TEMPERATURE = -1.0
PLACEHOLDER = -1
B, SPEC, V = 128, 8, 32000
NCORES = 8
RPC = B // NCORES      # 16 requests per core
HALF = V // 2          # 16000
CH = 4                 # vocab chunks per half for the streaming pass
CHW = HALF // CH       # 4000
SUB = V // 16          # 2000

_NC_CACHE = {}


def _build():
    import concourse.bass as bass
    import concourse.bacc as bacc
    import concourse.tile as tile
    from concourse import mybir

    F32 = mybir.dt.float32
    I32 = mybir.dt.int32
    U16 = mybir.dt.uint16
    U32 = mybir.dt.uint32
    AF = mybir.ActivationFunctionType
    OP = mybir.AluOpType
    AX = mybir.AxisListType

    nc = bacc.Bacc("TRN2", num_devices=8)

    lg_e = nc.declare_dram_parameter("lg", [128, HALF], F32, isOutput=False)
    lr_e = nc.declare_dram_parameter("lr", [128, HALF], F32, isOutput=False)
    dr_e = nc.declare_dram_parameter("dr", [64, V], F32, isOutput=False)
    qr_e = nc.declare_dram_parameter("qr", [8, V], F32, isOutput=False)
    u_e = nc.declare_dram_parameter("u_s", [64, 1], F32, isOutput=False)
    didg_e = nc.declare_dram_parameter("didg", [64, 1], F32, isOutput=False)
    didr8_e = nc.declare_dram_parameter("didr8", [8, 8], F32, isOutput=False)
    offs_e = nc.declare_dram_parameter("offs", [64, 1], I32, isOutput=False)
    valid9_e = nc.declare_dram_parameter("valid9", [9, 16], F32, isOutput=False)
    bonus_oh_e = nc.declare_dram_parameter("bonus_oh", [9, 16], F32, isOutput=False)
    bonusR_e = nc.declare_dram_parameter("bonusR", [9, 16], F32, isOutput=False)
    U9_e = nc.declare_dram_parameter("U9", [8, 9], F32, isOutput=False)
    miota_e = nc.declare_dram_parameter("miota", [8, 1], F32, isOutput=False)
    ones8_e = nc.declare_dram_parameter("ones8", [8, 1], F32, isOutput=False)
    jiota16_e = nc.declare_dram_parameter("jiota16", [8, 16], F32, isOutput=False)
    off2000_e = nc.declare_dram_parameter("off2000", [128, 1], F32, isOutput=False)
    pdiv16_e = nc.declare_dram_parameter("pdiv16", [128, 1], U16, isOutput=False)
    pm16_e = nc.declare_dram_parameter("pm16", [128, 1], F32, isOutput=False)
    riota8_e = nc.declare_dram_parameter("riota8", [1, 8], F32, isOutput=False)
    choff_e = nc.declare_dram_parameter("choff", [128, CH], F32, isOutput=False)
    out_e = nc.declare_dram_parameter("out", [9, 16], I32, isOutput=True)

    lr_flat1 = lr_e.rearrange("a b -> (a b)").rearrange("(r s) -> r s", s=1)
    dr_flat1 = dr_e.rearrange("a b -> (a b)").rearrange("(r s) -> r s", s=1)
    lr_sub = lr_e.rearrange("a b -> (a b)").rearrange("(r s) -> r s", s=SUB)
    dr_sub = dr_e.rearrange("a b -> (a b)").rearrange("(r s) -> r s", s=SUB)

    with tile.TileContext(nc) as tc, ExitStack() as ctx:
        const = ctx.enter_context(tc.tile_pool(name="const", bufs=1))
        big = ctx.enter_context(tc.tile_pool(name="big", bufs=3))
        lrp = ctx.enter_context(tc.tile_pool(name="lrp", bufs=2))
        keepp = ctx.enter_context(tc.tile_pool(name="keepp", bufs=1))
        small = ctx.enter_context(tc.tile_pool(name="small", bufs=1))
        psum = ctx.enter_context(tc.tile_pool(name="psum", bufs=1, space="PSUM"))

        def cload(ext, shape, dtype=F32):
            t = const.tile(shape, dtype, tag=ext.name)
            nc.sync.dma_start(t[:], ext[:])
            return t

        u_sb = cload(u_e, [64, 1])
        didg_sb = cload(didg_e, [64, 1])
        didr8_sb = cload(didr8_e, [8, 8])
        offs_sb = cload(offs_e, [64, 1], I32)
        valid9_sb = cload(valid9_e, [9, 16])
        bonus_oh_sb = cload(bonus_oh_e, [9, 16])
        bonusR_sb = cload(bonusR_e, [9, 16])
        U9_sb = cload(U9_e, [8, 9])
        miota_sb = cload(miota_e, [8, 1])
        ones8_sb = cload(ones8_e, [8, 1])
        jiota16_sb = cload(jiota16_e, [8, 16])
        off2000_sb = cload(off2000_e, [128, 1])
        pdiv16_sb = cload(pdiv16_e, [128, 1], U16)
        pm16_sb = cload(pm16_e, [128, 1])
        riota8_sb = cload(riota8_e, [1, 8])
        choff_sb = cload(choff_e, [128, CH])

        # q resident + -1/|q| via exp(-ln) + one Newton step; sign separately
        qsb = keepp.tile([128, SUB], F32, tag="qsb")
        nc.sync.dma_start(qsb[:], qr_e.rearrange("a b -> (a b)").rearrange("(p s) -> p s", s=SUB)[:])
        qa = big.tile([128, CHW], F32, tag="big")
        nc.scalar.activation(qa[:, :SUB], qsb[:], AF.Abs)
        nc.vector.tensor_scalar(out=qa[:, :SUB], in0=qa[:, :SUB], scalar1=1e-38, scalar2=None, op0=OP.max)
        lnq = big.tile([128, CHW], F32, tag="big")
        nc.scalar.activation(lnq[:, :SUB], qa[:, :SUB], AF.Ln)
        r0 = keepp.tile([128, SUB], F32, tag="r0")
        nc.scalar.activation(r0[:], lnq[:, :SUB], AF.Exp, scale=-1.0)
        t0q = big.tile([128, CHW], F32, tag="big")
        nc.vector.tensor_tensor(out=t0q[:, :SUB], in0=qa[:, :SUB], in1=r0[:], op=OP.mult)
        rqn = keepp.tile([128, SUB], F32, tag="rqn")
        nc.vector.scalar_tensor_tensor(out=rqn[:], in0=t0q[:, :SUB], scalar=2.0, in1=r0[:], op0=OP.subtract, op1=OP.mult)
        sgn = keepp.tile([128, SUB], F32, tag="sgn")
        nc.vector.tensor_scalar(out=sgn[:], in0=qsb[:], scalar1=0.0, scalar2=2.0, op0=OP.is_gt, op1=OP.mult)
        nc.vector.tensor_scalar(out=sgn[:], in0=sgn[:], scalar1=1.0, scalar2=None, op0=OP.subtract)

        # early element gathers: target_logits / draft_probs at the draft ids
        ld_sb = small.tile([64, 1], F32, tag="ld")
        nc.gpsimd.indirect_dma_start(
            out=ld_sb[:], out_offset=None, in_=lr_flat1[:],
            in_offset=bass.IndirectOffsetOnAxis(ap=offs_sb[:, :1], axis=0))
        pd_sb = small.tile([64, 1], F32, tag="pd")
        nc.gpsimd.indirect_dma_start(
            out=pd_sb[:], out_offset=None, in_=dr_flat1[:],
            in_offset=bass.IndirectOffsetOnAxis(ap=offs_sb[:, :1], axis=0))

        # phase 1: stream vocab chunks; DVE max/argmax on greedy rows,
        # ACT exp+accumulate (softmax denominator) on non-greedy rows
        gmax8 = small.tile([128, 8 * CH], F32, tag="gmax8")
        gidx8 = small.tile([128, 8 * CH], U32, tag="gidx8")
        zacc = small.tile([128, CH], F32, tag="zacc")
        for k in range(CH):
            lgt = big.tile([128, CHW], F32, tag="big")
            nc.sync.dma_start(lgt[:], lg_e[:, k * CHW:(k + 1) * CHW])
            lrt = lrp.tile([128, CHW], F32, tag="lr")
            nc.sync.dma_start(lrt[:], lr_e[:, k * CHW:(k + 1) * CHW])
            nc.vector.max(out=gmax8[:, 8 * k:8 * k + 8], in_=lgt[:])
            nc.vector.max_index(gidx8[:, 8 * k:8 * k + 8], gmax8[:, 8 * k:8 * k + 8], lgt[:])
            nc.scalar.activation(lrt[:], lrt[:], AF.Exp, accum_out=zacc[:, k:k + 1])

        # per-partition argmax combine across chunks (first occurrence)
        gmaxv = gmax8[:, 0:8 * CH:8]
        gmax_p = small.tile([128, 1], F32, tag="gmax_p")
        nc.vector.tensor_reduce(out=gmax_p[:], in_=gmaxv, op=OP.max, axis=AX.X)
        eq = small.tile([128, CH], F32, tag="eq")
        nc.vector.tensor_scalar(out=eq[:], in0=gmaxv, scalar1=gmax_p[:], scalar2=None, op0=OP.is_equal)
        cum = small.tile([128, CH], F32, tag="cum")
        nc.vector.tensor_copy(cum[:], eq[:])
        nc.vector.tensor_tensor(out=cum[:, 1:4], in0=eq[:, 1:4], in1=eq[:, 0:3], op=OP.add)
        cum2 = small.tile([128, CH], F32, tag="cum2")
        nc.vector.tensor_copy(cum2[:], cum[:])
        nc.vector.tensor_tensor(out=cum2[:, 2:4], in0=cum[:, 2:4], in1=cum[:, 0:2], op=OP.add)
        firstm = small.tile([128, CH], F32, tag="firstm")
        nc.vector.tensor_scalar(out=firstm[:], in0=cum2[:], scalar1=1.0, scalar2=None, op0=OP.is_equal)
        nc.vector.tensor_tensor(out=firstm[:], in0=firstm[:], in1=eq[:], op=OP.mult)
        idxf = small.tile([128, CH], F32, tag="idxf")
        nc.vector.tensor_copy(idxf[:], gidx8[:, 0:8 * CH:8])
        nc.vector.tensor_tensor(out=idxf[:], in0=idxf[:], in1=choff_sb[:], op=OP.add)
        nc.vector.tensor_tensor(out=idxf[:], in0=idxf[:], in1=firstm[:], op=OP.mult)
        parg = small.tile([128, 1], F32, tag="parg")
        nc.vector.tensor_reduce(out=parg[:], in_=idxf[:], op=OP.add, axis=AX.X)

        # combine the two half-row partitions of each greedy slot
        pk = small.tile([128, 2], F32, tag="pk")
        nc.vector.tensor_copy(pk[:, 0:1], gmax_p[:])
        nc.vector.tensor_copy(pk[:, 1:2], parg[:])
        pk2 = small.tile([64, 4], F32, tag="pk2")
        nc.sync.dma_start(pk2[:], pk[:])
        hsel = small.tile([64, 1], F32, tag="hsel")
        nc.vector.tensor_tensor(out=hsel[:], in0=pk2[:, 2:3], in1=pk2[:, 0:1], op=OP.is_gt)
        amx = small.tile([64, 1], F32, tag="amx")
        nc.vector.tensor_scalar(out=amx[:], in0=pk2[:, 3:4], scalar1=float(HALF), scalar2=None, op0=OP.add)
        nc.vector.tensor_tensor(out=amx[:], in0=amx[:], in1=pk2[:, 1:2], op=OP.subtract)
        nc.vector.tensor_tensor(out=amx[:], in0=amx[:], in1=hsel[:], op=OP.mult)
        nc.vector.tensor_tensor(out=amx[:], in0=amx[:], in1=pk2[:, 1:2], op=OP.add)
        acc_g = small.tile([64, 1], F32, tag="acc_g")
        nc.vector.tensor_tensor(out=acc_g[:], in0=didg_sb[:], in1=amx[:], op=OP.is_equal)

        # softmax denominators per slot; acceptance tests
        zsum = small.tile([128, 1], F32, tag="zsum")
        nc.vector.tensor_reduce(out=zsum[:], in_=zacc[:], op=OP.add, axis=AX.X)
        z2 = small.tile([64, 2], F32, tag="z2")
        nc.sync.dma_start(z2[:], zsum[:])
        Zs = small.tile([64, 1], F32, tag="Zs")
        nc.vector.tensor_reduce(out=Zs[:], in_=z2[:], op=OP.add, axis=AX.X)
        rz = small.tile([64, 1], F32, tag="rz")
        nc.vector.reciprocal(rz[:], Zs[:])
        eld = small.tile([64, 1], F32, tag="eld")
        nc.scalar.activation(eld[:], ld_sb[:], AF.Exp)
        ptgt = small.tile([64, 1], F32, tag="ptgt")
        nc.vector.tensor_tensor(out=ptgt[:], in0=eld[:], in1=rz[:], op=OP.mult)
        upd = small.tile([64, 1], F32, tag="upd")
        nc.vector.tensor_tensor(out=upd[:], in0=u_sb[:], in1=pd_sb[:], op=OP.mult)
        acc_r = small.tile([64, 1], F32, tag="acc_r")
        nc.vector.tensor_tensor(out=acc_r[:], in0=ptgt[:], in1=upd[:], op=OP.is_ge)
        pdpos = small.tile([64, 1], F32, tag="pdpos")
        nc.vector.tensor_scalar(out=pdpos[:], in0=pd_sb[:], scalar1=0.0, scalar2=None, op0=OP.is_gt)
        nc.vector.tensor_tensor(out=acc_r[:], in0=acc_r[:], in1=pdpos[:], op=OP.mult)

        # rejection-prefix logic over [8 pos, 16 req]
        acc16 = small.tile([8, 16], F32, tag="acc16")
        nc.sync.dma_start(acc16[:, 0:8], acc_g[:])
        nc.sync.dma_start(acc16[:, 8:16], acc_r[:])
        rejN = small.tile([8, 16], F32, tag="rejN")
        nc.vector.scalar_tensor_tensor(out=rejN[:], in0=acc16[:], scalar=1.0, in1=valid9_sb[0:8, :], op0=OP.subtract, op1=OP.mult)
        rb_ps = psum.tile([9, 16], F32, tag="rb", space="PSUM")
        nc.tensor.matmul(rb_ps[:], lhsT=U9_sb[:], rhs=rejN[:], start=True, stop=True)
        keep = small.tile([9, 16], F32, tag="keep")
        nc.vector.tensor_scalar(out=keep[:], in0=rb_ps[:], scalar1=0.0, scalar2=None, op0=OP.is_equal)
        fr = small.tile([8, 16], F32, tag="fr")
        nc.vector.tensor_tensor(out=fr[:], in0=keep[0:8, :], in1=rejN[:], op=OP.mult)
        nc.vector.tensor_scalar(out=fr[:], in0=fr[:], scalar1=-1.0, scalar2=None, op0=OP.mult)
        accM = small.tile([8, 8], F32, tag="accM")
        nc.vector.tensor_scalar(out=accM[:], in0=rejN[:, 8:16], scalar1=1.0, scalar2=None, op0=OP.add)

        sp_ps = psum.tile([1, 16], F32, tag="sp", space="PSUM")
        nc.tensor.matmul(sp_ps[:], lhsT=miota_sb[:], rhs=fr[:], start=True, stop=True)
        offs0 = small.tile([1, 8], F32, tag="offs0")
        nc.vector.scalar_tensor_tensor(out=offs0[:], in0=sp_ps[0:1, 8:16], scalar=8.0, in1=riota8_sb[:], op0=OP.mult, op1=OP.add)

        Zs8 = small.tile([8, 8], F32, tag="Zs8")
        nc.sync.dma_start(Zs8[:], Zs[:])
        zfr = small.tile([8, 8], F32, tag="zfr")
        nc.vector.tensor_tensor(out=zfr[:], in0=Zs8[:], in1=fr[:, 8:16], op=OP.mult)
        zs_ps = psum.tile([1, 8], F32, tag="zsps", space="PSUM")
        nc.tensor.matmul(zs_ps[:], lhsT=ones8_sb[:], rhs=zfr[:], start=True, stop=True)
        zrow = small.tile([1, 8], F32, tag="zrow")
        nc.vector.tensor_copy(zrow[:], zs_ps[:])

        # per-16-partition-group broadcast of the selected row / its Z
        offs0bc = small.tile([128, 8], F32, tag="offs0bc")
        nc.gpsimd.partition_broadcast(offs0bc[:], offs0[:])
        selv = small.tile([128, 1], F32, tag="selv")
        nc.gpsimd.indirect_copy(out=selv[:], data=offs0bc[:], idxs=pdiv16_sb[:], i_know_ap_gather_is_preferred=True)
        zbc8 = small.tile([128, 8], F32, tag="zbc8")
        nc.gpsimd.partition_broadcast(zbc8[:], zrow[:])
        Zbc = small.tile([128, 1], F32, tag="Zbc")
        nc.gpsimd.indirect_copy(out=Zbc[:], data=zbc8[:], idxs=pdiv16_sb[:], i_know_ap_gather_is_preferred=True)

        offsub_f = small.tile([128, 1], F32, tag="offsub_f")
        nc.vector.scalar_tensor_tensor(out=offsub_f[:], in0=selv[:], scalar=16.0, in1=pm16_sb[:], op0=OP.mult, op1=OP.add)
        offsub_i = small.tile([128, 1], I32, tag="offsub_i")
        nc.vector.tensor_copy(offsub_i[:], offsub_f[:])

        # gather the first-rejection row of each non-greedy request
        lsel = big.tile([128, CHW], F32, tag="big")
        nc.gpsimd.indirect_dma_start(
            out=lsel[:, :SUB], out_offset=None, in_=lr_sub[:],
            in_offset=bass.IndirectOffsetOnAxis(ap=offsub_i[:, :1], axis=0))
        dsel = big.tile([128, CHW], F32, tag="big")
        nc.gpsimd.indirect_dma_start(
            out=dsel[:, :SUB], out_offset=None, in_=dr_sub[:],
            in_offset=bass.IndirectOffsetOnAxis(ap=offsub_i[:, :1], axis=0))

        # recovered-token argmax over (exp(l) - Z d) / q  (sign-correct for q<=0)
        esel = big.tile([128, CHW], F32, tag="big")
        nc.scalar.activation(esel[:, :SUB], lsel[:, :SUB], AF.Exp)
        zd = lrp.tile([128, CHW], F32, tag="lr")
        nc.vector.tensor_scalar(out=zd[:, :SUB], in0=dsel[:, :SUB], scalar1=Zbc[:], scalar2=None, op0=OP.mult)
        s_t = lrp.tile([128, CHW], F32, tag="lr")
        nc.vector.tensor_tensor(out=s_t[:, :SUB], in0=zd[:, :SUB], in1=esel[:, :SUB], op=OP.subtract)
        nc.vector.tensor_tensor(out=s_t[:, :SUB], in0=s_t[:, :SUB], in1=sgn[:], op=OP.mult)
        adj = big.tile([128, CHW], F32, tag="big")
        nc.vector.tensor_tensor(out=adj[:, :SUB], in0=s_t[:, :SUB], in1=rqn[:], op=OP.mult)
        rmax8 = small.tile([128, 8], F32, tag="rmax8")
        nc.vector.max(out=rmax8[:], in_=adj[:, :SUB])
        ridx8 = small.tile([128, 8], U32, tag="ridx8")
        nc.vector.max_index(ridx8[:], rmax8[:], adj[:, :SUB])
        rpk = small.tile([128, 2], F32, tag="rpk")
        nc.vector.tensor_copy(rpk[:, 1:2], ridx8[:, 0:1])
        nc.vector.tensor_tensor(out=rpk[:, 1:2], in0=rpk[:, 1:2], in1=off2000_sb[:], op=OP.add)
        nc.vector.tensor_copy(rpk[:, 0:1], rmax8[:, 0:1])
        rpk16 = small.tile([8, 32], F32, tag="rpk16")
        nc.sync.dma_start(rpk16[:], rpk[:])
        jm8 = small.tile([8, 8], F32, tag="jm8")
        nc.vector.max(out=jm8[:], in_=rpk16[:, 0:32:2])
        jidx8 = small.tile([8, 8], U32, tag="jidx8")
        nc.vector.max_index(jidx8[:], jm8[:], rpk16[:, 0:32:2])
        jsf = small.tile([8, 1], F32, tag="jsf")
        nc.vector.tensor_copy(jsf[:], jidx8[:, 0:1])
        msel = small.tile([8, 16], F32, tag="msel")
        nc.vector.tensor_scalar(out=msel[:], in0=jiota16_sb[:], scalar1=jsf[:], scalar2=None, op0=OP.is_equal)
        nc.vector.tensor_tensor(out=msel[:], in0=msel[:], in1=rpk16[:, 1:32:2], op=OP.mult)
        recov8 = small.tile([8, 1], F32, tag="recov8")
        nc.vector.tensor_reduce(out=recov8[:], in_=msel[:], op=OP.add, axis=AX.X)
        recovrow = small.tile([1, 8], F32, tag="recovrow")
        nc.sync.dma_start(recovrow[:], recov8[:])
        ones18 = small.tile([1, 8], F32, tag="ones18")
        nc.vector.memset(ones18[:], 1.0)
        rb2_ps = psum.tile([8, 8], F32, tag="rbc", space="PSUM")
        nc.tensor.matmul(rb2_ps[:], lhsT=ones18[:], rhs=recovrow[:], start=True, stop=True)

        # final assembly of the [9 pos, 16 req] output
        cand = small.tile([9, 16], F32, tag="cand")
        nc.vector.memset(cand[:], 0.0)
        nc.sync.dma_start(cand[0:8, 0:8], amx[:])
        t1 = small.tile([8, 8], F32, tag="t1")
        nc.vector.tensor_tensor(out=t1[:], in0=accM[:], in1=didr8_sb[:], op=OP.mult)
        invA = small.tile([8, 8], F32, tag="invA")
        nc.vector.tensor_scalar(out=invA[:], in0=accM[:], scalar1=-1.0, scalar2=1.0, op0=OP.mult, op1=OP.add)
        nc.vector.tensor_tensor(out=invA[:], in0=invA[:], in1=rb2_ps[:], op=OP.mult)
        nc.vector.tensor_tensor(out=cand[0:8, 8:16], in0=t1[:], in1=invA[:], op=OP.add)

        w1 = small.tile([9, 16], F32, tag="w1")
        nc.vector.tensor_tensor(out=w1[:], in0=keep[:], in1=valid9_sb[:], op=OP.mult)
        w2 = small.tile([9, 16], F32, tag="w2")
        nc.vector.tensor_tensor(out=w2[:], in0=keep[:], in1=bonus_oh_sb[:], op=OP.mult)
        outf = small.tile([9, 16], F32, tag="outf")
        nc.vector.tensor_tensor(out=outf[:], in0=w1[:], in1=cand[:], op=OP.mult)
        ob = small.tile([9, 16], F32, tag="ob")
        nc.vector.tensor_tensor(out=ob[:], in0=w2[:], in1=bonusR_sb[:], op=OP.mult)
        nc.vector.tensor_tensor(out=outf[:], in0=outf[:], in1=ob[:], op=OP.add)
        nc.vector.tensor_tensor(out=outf[:], in0=outf[:], in1=w1[:], op=OP.add)
        nc.vector.tensor_tensor(out=outf[:], in0=outf[:], in1=w2[:], op=OP.add)
        nc.vector.tensor_scalar(out=outf[:], in0=outf[:], scalar1=1.0, scalar2=None, op0=OP.subtract)
        outi = small.tile([9, 16], I32, tag="outi")
        nc.vector.tensor_copy(outi[:], outf[:])
        nc.sync.dma_start(out_e[:], outi[:])

    nc.compile()
    return nc


def _host_prepare(inputs):
    dp = np.ascontiguousarray(np.asarray(inputs["draft_probs"], np.float32))
    tl = np.ascontiguousarray(np.asarray(inputs["target_logits"], np.float32))
    q = np.ascontiguousarray(np.asarray(inputs["q"], np.float32))
    u = np.asarray(inputs["uniform_probs"], np.float32)
    temp = np.asarray(inputs["temperature"], np.float32)
    did = np.asarray(inputs["draft_token_ids"], np.int32)
    cu = np.asarray(inputs["cu_num_draft_tokens"], np.int64)
    bonus = np.asarray(inputs["bonus_token_ids"], np.int32)

    prev_cu = np.concatenate([[0], cu[:-1]])
    nd = cu - prev_cu
    if nd.min() < 0 or nd.max() > SPEC:
        raise _FallbackNeeded()
    is_greedy = temp == GREEDY_TEMPERATURE

    pgrid = np.arange(SPEC)                       # positions

    in_maps = []
    colmaps = []
    for c in range(NCORES):
        reqs = np.arange(c * RPC, (c + 1) * RPC)
        G = reqs[is_greedy[reqs]]
        R = reqs[~is_greedy[reqs]]
        if len(G) != 8 or len(R) != 8:
            raise _FallbackNeeded()
        cols = np.concatenate([G, R])
        colmaps.append(cols)

        # slot s = 8*p + j (pos-major); token index or -1 for padding
        tokG = prev_cu[G][None, :] + pgrid[:, None]        # [8 pos, 8 j]
        mG = pgrid[:, None] < nd[G][None, :]
        tokG = np.where(mG, tokG, -1).reshape(64)
        tokR = prev_cu[R][None, :] + pgrid[:, None]
        mR = pgrid[:, None] < nd[R][None, :]
        tokR = np.where(mR, tokR, -1).reshape(64)

        lgrows = np.where((tokG >= 0)[:, None], tl[tokG.clip(0)], 0.0)
        lrrows = np.where((tokR >= 0)[:, None], tl[tokR.clip(0)], 0.0)
        drrows = np.where((tokR >= 0)[:, None], dp[tokR.clip(0)], 0.0)

        didg = np.where(tokG >= 0, did[tokG.clip(0)], 0).astype(np.float32)
        didr = np.where(tokR >= 0, did[tokR.clip(0)], 0)
        u_s = np.where(tokR >= 0, u[tokR.clip(0)], 0.0).astype(np.float32)
        offs = (np.arange(64) * V + didr).astype(np.int32)

        valid9 = (np.arange(9)[:, None] < nd[cols][None, :]).astype(np.float32)
        bonus_oh = (np.arange(9)[:, None] == nd[cols][None, :]).astype(np.float32)
        bonusR = np.broadcast_to(bonus[cols, 0].astype(np.float32)[None, :], (9, 16)).copy()
        U9 = np.triu(np.ones((9, 9), np.float32), 1)[:8]

        in_maps.append({
            "lg": lgrows.reshape(128, HALF),
            "lr": lrrows.reshape(128, HALF),
            "dr": drrows,
            "qr": np.ascontiguousarray(q[R]),
            "u_s": u_s[:, None],
            "didg": didg[:, None],
            "didr8": didr.astype(np.float32).reshape(8, 8),
            "offs": offs[:, None],
            "valid9": valid9,
            "bonus_oh": bonus_oh,
            "bonusR": bonusR,
            "U9": U9,
            "miota": np.arange(8, dtype=np.float32)[:, None],
            "ones8": np.ones((8, 1), np.float32),
            "jiota16": np.tile(np.arange(16, dtype=np.float32), (8, 1)),
            "off2000": (np.arange(128, dtype=np.float32) % 16 * SUB)[:, None],
            "pdiv16": (np.arange(128) // 16).astype(np.uint16)[:, None],
            "pm16": (np.arange(128) % 16).astype(np.float32)[:, None],
            "riota8": np.arange(8, dtype=np.float32)[None, :],
            "choff": np.tile((np.arange(CH) * CHW).astype(np.float32), (128, 1)),
        })
    return in_maps, colmaps


class _FallbackNeeded(Exception):
    pass


def _numpy_reference(inputs):
    """Pure-numpy port of the reference; fallback for unexpected input shapes."""
    dp = np.asarray(inputs["draft_probs"], np.float32)
    tl = np.asarray(inputs["target_logits"], np.float32)
    q = np.asarray(inputs["q"], np.float32)
    u = np.asarray(inputs["uniform_probs"], np.float32)
    temp = np.asarray(inputs["temperature"], np.float32)
    did = np.asarray(inputs["draft_token_ids"], np.int32)
    cu = np.asarray(inputs["cu_num_draft_tokens"], np.int64)
    bonus = np.asarray(inputs["bonus_token_ids"], np.int32)
    msl = int(np.asarray(inputs["max_spec_len"]))
    n = did.shape[0]
    b = cu.shape[0]
    x = tl - tl.max(axis=-1, keepdims=True)
    e = np.exp(x)
    tp = e / e.sum(axis=-1, keepdims=True)
    tok = np.arange(n)
    req = np.searchsorted(cu, tok, side="right")
    prev = np.concatenate([[0], cu[:-1]])
    start = prev[req]
    pos = tok - start
    g = (temp == GREEDY_TEMPERATURE)[req]
    am = tp.argmax(axis=-1).astype(np.int32)
    accg = did == am
    pd = dp[tok, did]
    pt = tp[tok, did]
    accr = (pd > 0) & (pt >= u * pd)
    adjusted = np.maximum(tp - dp, 0.0)
    rec = (adjusted / q[req]).argmax(axis=-1).astype(np.int32)
    acc = np.where(g, accg, accr)
    token = np.where(g, am, np.where(accr, did, rec))
    rej = (~acc).astype(np.int64)
    cs = rej.cumsum()
    seg = np.where(start > 0, cs[(start - 1).clip(0)], 0)
    keep = (cs - seg - rej) == 0
    out = np.full((b, msl + 1), PLACEHOLDER, np.int32)
    out[req, pos] = np.where(keep, token, PLACEHOLDER)
    ndr = cu - prev
    segrej = np.zeros(b, np.int64)
    np.add.at(segrej, req, rej)
    out[np.arange(b), ndr] = np.where(segrej == 0, bonus[:, 0], PLACEHOLDER)
    return out


def _run_on_cores(in_maps, trace=False):
    from concourse.bass_utils import run_bass_kernel_spmd
    if "nc" not in _NC_CACHE:
        _NC_CACHE["nc"] = _build()
    nc = _NC_CACHE["nc"]
    res = run_bass_kernel_spmd(nc, in_maps, core_ids=list(range(NCORES)), trace=trace)
    return res


def kernel(**inputs) -> np.ndarray:
    try:
        in_maps, colmaps = _host_prepare(inputs)
    except _FallbackNeeded:
        return _numpy_reference(inputs)
    res = _run_on_cores(in_maps, trace=False)
    out = np.full((B, SPEC + 1), PLACEHOLDER, np.int32)
    for c in range(NCORES):
        o = res.results[c]["out"]                # [9, 16] int32
        out[colmaps[c]] = o.T
    return out
